# revision 22
# baseline (speedup 1.0000x reference)
"""Trainium2 Bass kernel for nn_EDTransformer (encoder-decoder transformer).

Sharding: 8 cores = 4 batch items x 2 sequence halves.
 - Each core owns (item b, half h): computes Q/scores/AV/Wo/MLP/LN for its
   256 local positions, K/V redundantly for the full 512 positions.
 - One 2-core AllGather of fp8 activations per layer boundary (3 total).
 - The decoder's first self-attention block is encoder-independent; its
   pieces are emitted interleaved into the encoder's LN/AllGather bubbles
   so the tensor engine never idles there.
 - Unembedding fully local (transposed): each core computes logits^T
   [tokens, vocab] for its 256 positions over the full 32000 vocab with
   Wu streamed once in fp16; softmax denominator via Exp accum_out,
   normalization split across scalar+vector; no global collectives.
Dtypes: fp8e4 DoubleRow matmuls for Q/K/V only (x64 weight prescale;
 error-sim shows ~1e-2 rel err, 2x margin); fp16 for scores/AV/Wo/MLP/
 unembed; fp32 PSUM; residual stream fp32 kept at 64x true scale so
 Wo/W2 outputs add with no extra rescale ops; fp16 output.
"""
import os
import sys

sys.path.insert(0, '/opt/trn_rl_repo')
import numpy as np
import ml_dtypes

import concourse.bacc as bacc
import concourse.tile as tile
import concourse.mybir as mybir
from concourse.bass_utils import run_bass_kernel_spmd

DT = mybir.dt
F8 = DT.float8e4
F16 = DT.float16
F32 = DT.float32
NPF8 = ml_dtypes.float8_e4m3
DR = mybir.MatmulPerfMode.DoubleRow
AF = mybir.ActivationFunctionType
MUL = mybir.AluOpType.mult
ADD = mybir.AluOpType.add

N_CORES = 8
P = 128
DE = 1024          # model dim        (8 ptiles)
KO = DE // P       # 8
KP = KO // 2       # 4 fp8 k-pairs
DMLP = 4096        # mlp dim          (32 ptiles)
MO = DMLP // P     # 32
H = 16             # heads
DA = 64            # attn dim per head
L = 512            # sequence length
LL = 256           # local positions per core
KT = L // P        # 4 kz tiles
NV = 32000
NCH = 63           # unembed vocab chunks: 62 x 512 + 1 x 256
LENC = 2
LDEC = 2
EPS = 1e-5

SW = 64.0          # fp8 weight prescale
SR = 64.0          # residual stream scale (Eres = SR * true)
SIN0 = 32.0        # fp8 scale of the raw (pre-LN) embedding stream
S_RAW = 1.0 / (SW * SIN0)   # psum unscale when stream input is raw
S_LN = 1.0 / SW             # psum unscale when stream input is post-LN

PAIR_GROUPS = [[0, 1], [2, 3], [4, 5], [6, 7]]

_CACHE = {}


# ----------------------------------------------------------------------------
# attention pieces
# ----------------------------------------------------------------------------

def _attn_q(nc, pools, qin8, wq_d, s_q):
    """Q projection: qin8 [128, KO, LL] fp8 -> q16 [128, KO, LL]."""
    p256 = pools['p256']
    q16 = pools['att'].tile([P, KO, LL], F16, tag='q16', name='q16')
    for pr in range(KO):
        wt = pools['wqp'].tile([P, KO, P], F8, tag='wqt', name='wqt')
        nc.sync.dma_start(wt[:], wq_d[pr])
        ps = p256.tile([P, LL], F32, tag='p256', name='psq')
        for k in range(KP):
            nc.tensor.matmul(ps[:], wt[:, 2 * k:2 * k + 2, :],
                             qin8[:, 2 * k:2 * k + 2, :],
                             start=(k == 0), stop=(k == KP - 1), perf_mode=DR)
        nc.scalar.activation(q16[:, pr, :], ps[:], AF.Copy, scale=s_q)
    return q16


def _attn_kv(nc, pools, kvin8, wk_d, wv_d, s_kv):
    """K/V projections over the full sequence.
    k16 [128, KO(pr), L]; vt16 [128(kz), KT, H*DA]."""
    p512 = pools['p512']
    k16 = pools['att'].tile([P, KO, L], F16, tag='k16', name='k16')
    for pr in range(KO):
        wt = pools['wkp'].tile([P, KO, P], F8, tag='wkt', name='wkt')
        nc.sync.dma_start(wt[:], wk_d[pr])
        ps = p512.tile([P, L], F32, tag='p512', name='psk')
        for k in range(KP):
            nc.tensor.matmul(ps[:], wt[:, 2 * k:2 * k + 2, :],
                             kvin8[:, 2 * k:2 * k + 2, :],
                             start=(k == 0), stop=(k == KP - 1), perf_mode=DR)
        nc.scalar.activation(k16[:, pr, :], ps[:], AF.Copy, scale=s_kv)
    vt16 = pools['att'].tile([P, KT, H * DA], F16, tag='vt16', name='vt16')
    for nch in range(2):
        wt = pools['wvp'].tile([P, KO, 512], F8, tag='wvt', name='wvt')
        nc.sync.dma_start(wt[:], wv_d[nch])
        for kt in range(KT):
            ps = p512.tile([P, 512], F32, tag='p512', name='psv')
            for k in range(KP):
                nc.tensor.matmul(ps[:],
                                 kvin8[:, 2 * k:2 * k + 2, kt * P:(kt + 1) * P],
                                 wt[:, 2 * k:2 * k + 2, :],
                                 start=(k == 0), stop=(k == KP - 1),
                                 perf_mode=DR)
            nc.scalar.activation(vt16[:, kt, nch * 512:(nch + 1) * 512], ps[:],
                                 AF.Copy, scale=s_kv)
    return k16, vt16


def _attn_kv_local(nc, pools, ein8, wk_d, wv_d, s_kv):
    """K/V over the LOCAL 256 positions only (for the KV-AllGather scheme).
    kloc [128, KO(pr), LL]; vloc [128(kz-local), 2, H*DA]."""
    p256 = pools['p256']
    p512 = pools['p512']
    kloc = pools['att'].tile([P, KO, LL], F16, tag='kloc', name='kloc')
    for pr in range(KO):
        wt = pools['wkp'].tile([P, KO, P], F8, tag='wkt', name='wkt')
        nc.sync.dma_start(wt[:], wk_d[pr])
        ps = p256.tile([P, LL], F32, tag='p256', name='pskl')
        for k in range(KP):
            nc.tensor.matmul(ps[:], wt[:, 2 * k:2 * k + 2, :],
                             ein8[:, 2 * k:2 * k + 2, :],
                             start=(k == 0), stop=(k == KP - 1), perf_mode=DR)
        nc.scalar.activation(kloc[:, pr, :], ps[:], AF.Copy, scale=s_kv)
    vloc = pools['att'].tile([P, 2, H * DA], F16, tag='vloc', name='vloc')
    for nch in range(2):
        wt = pools['wvp'].tile([P, KO, 512], F8, tag='wvt', name='wvt')
        nc.sync.dma_start(wt[:], wv_d[nch])
        for ktl in range(2):
            ps = p512.tile([P, 512], F32, tag='p512', name='psvl')
            for k in range(KP):
                nc.tensor.matmul(ps[:],
                                 ein8[:, 2 * k:2 * k + 2,
                                      ktl * P:(ktl + 1) * P],
                                 wt[:, 2 * k:2 * k + 2, :],
                                 start=(k == 0), stop=(k == KP - 1),
                                 perf_mode=DR)
            nc.scalar.activation(vloc[:, ktl, nch * 512:(nch + 1) * 512],
                                 ps[:], AF.Copy, scale=s_kv)
    return kloc, vloc


def _kv_ag_issue(nc, kloc, vloc, agkin, agkout):
    """Stage local K/V and issue the pair AllGather."""
    nc.sync.dma_start(
        agkin[:, :KO * LL].rearrange('ki (pr c) -> ki pr c', pr=KO), kloc[:])
    nc.sync.dma_start(
        agkin[:, KO * LL:].rearrange('ki (ktl o) -> ki ktl o', ktl=2),
        vloc[:])
    nc.gpsimd.collective_compute(
        "AllGather", mybir.AluOpType.bypass,
        ins=[agkin[:]], outs=[agkout[:]],
        replica_groups=PAIR_GROUPS)


def _kv_ag_assemble(nc, pools, agkout, ktag='k16', vtag='vt16'):
    """Assemble full-sequence k16/vt16 from the AllGather output."""
    k16 = pools['att'].tile([P, KO, L], F16, tag=ktag, name='k16a')
    for pr in range(KO):
        nc.sync.dma_start(
            k16[:, pr, :].rearrange('ki (r c) -> ki r c', r=2),
            agkout[:, :, pr * LL:(pr + 1) * LL].rearrange(
                'r ki c -> ki r c'))
    vt16 = pools['att'].tile([P, KT, H * DA], F16, tag=vtag, name='vt16a')
    nc.sync.dma_start(
        vt16[:].rearrange('ki (r ktl) o -> ki r ktl o', r=2),
        agkout[:, :, KO * LL:].rearrange('r ki (ktl o) -> ki r ktl o', ktl=2))
    return k16, vt16


def _attn_sav(nc, pools, q16, k16, vt16, mask):
    """scores -> exp -> (mask) -> deno/AV pipelined per head-pair.
    Returns y16 [128, KO(pr), LL] at 64x true scale."""
    sb = pools['att']
    p256 = pools['p256']
    p512 = pools['p512']
    ones64 = pools['ones64']

    exp16 = sb.tile([P, H, KT, LL], F16, tag='exp16', name='exp16')
    y16 = sb.tile([P, KO, LL], F16, tag='y16', name='y16')
    yscs = []

    def emit_av(pr):
        hA, hB = 2 * pr, 2 * pr + 1
        ysc = yscs[pr]
        ps = p256.tile([P, LL], F32, tag='p256', name='psav')
        for kt in range(KT):
            nc.tensor.matmul(ps[:DA, :], vt16[:, kt, hA * DA:(hA + 1) * DA],
                             exp16[:, hA, kt, :], start=(kt == 0),
                             stop=(kt == KT - 1), tile_position=(0, 0))
            nc.tensor.matmul(ps[DA:, :], vt16[:, kt, hB * DA:(hB + 1) * DA],
                             exp16[:, hB, kt, :], start=(kt == 0),
                             stop=(kt == KT - 1), tile_position=(0, DA))
        nc.vector.tensor_tensor(y16[:DA, pr, :], ps[:DA, :], ysc[:DA, 0, :],
                                MUL)
        nc.vector.tensor_tensor(y16[DA:, pr, :], ps[DA:, :], ysc[DA:, 1, :],
                                MUL)

    for pr in range(KO):
        hA, hB = 2 * pr, 2 * pr + 1
        for h in (hA, hB):
            hp = (h % 2) * DA
            for kt in range(KT):
                ps = p256.tile([P, LL], F32, tag='p256', name='pssc')
                nc.tensor.matmul(ps[:],
                                 k16[hp:hp + DA, pr, kt * P:(kt + 1) * P],
                                 q16[hp:hp + DA, pr, :], start=True, stop=True)
                nc.scalar.activation(exp16[:, h, kt, :], ps[:], AF.Exp)
        if mask is not None:
            for kt in range(KT):
                nc.vector.tensor_tensor(
                    exp16[:, hA:hB + 1, kt, :], exp16[:, hA:hB + 1, kt, :],
                    mask[:, kt, None, :].to_broadcast((P, 2, LL)), MUL)
        pd = p512.tile([P, 2 * LL], F32, tag='p512', name='psd')
        for kt in range(KT):
            nc.tensor.matmul(pd[:], ones64[:, :],
                             exp16[:, hA:hB + 1, kt, :],
                             start=(kt == 0), stop=(kt == KT - 1))
        ysc = sb.tile([P, 2, LL], F32, tag='ysc', name='ysc', bufs=2)
        nc.vector.reciprocal(ysc[:], pd[:])
        yscs.append(ysc)
        if pr > 0:
            emit_av(pr - 1)
    emit_av(KO - 1)
    return exp16, y16


def _attn_wo(nc, pools, y16, wo_d, Eres32):
    """Wo (fp16 true scale): psum at 64x true -> accumulate into residual."""
    p256 = pools['p256']
    for dt in range(KO):
        wt = pools['wop'].tile([P, KO, P], F16, tag='wot', name='wot')
        nc.sync.dma_start(wt[:], wo_d[dt])
        ps = p256.tile([P, LL], F32, tag='p256', name='pswo')
        for k in range(KO):
            nc.tensor.matmul(ps[:], wt[:, k, :],
                             y16[:, k, :], start=(k == 0), stop=(k == KO - 1))
        nc.vector.tensor_tensor(Eres32[:, dt, :], Eres32[:, dt, :], ps[:],
                                ADD)


def _mlp(nc, pools, Eres32, ein16, w1_dram, w2_dram):
    """ein16 true scale fp16; w1 true fp16; w2 pre-scaled x64 fp16."""
    p256 = pools['p256']
    h16 = pools['mlp'].tile([P, MO, LL], F16, tag='h16', name='h16')
    for mt in range(MO):
        w1t = pools['w1p'].tile([P, KO, P], F16, tag='w1t', name='w1t')
        nc.sync.dma_start(w1t[:], w1_dram[mt])
        ps = p256.tile([P, LL], F32, tag='p256', name='psw1')
        for k in range(KO):
            nc.tensor.matmul(ps[:], w1t[:, k, :], ein16[:, k, :],
                             start=(k == 0), stop=(k == KO - 1))
        nc.scalar.activation(h16[:, mt, :], ps[:], AF.Relu)
    for dt in range(KO):
        w2t = pools['w2p'].tile([P, MO, P], F16, tag='w2t', name='w2t')
        nc.sync.dma_start(w2t[:], w2_dram[dt])
        ps = p256.tile([P, LL], F32, tag='p256', name='psw2')
        for k in range(MO):
            nc.tensor.matmul(ps[:], w2t[:, k, :], h16[:, k, :],
                             start=(k == 0), stop=(k == MO - 1))
        nc.vector.tensor_tensor(Eres32[:, dt, :], Eres32[:, dt, :], ps[:],
                                ADD)


def _ln(nc, pools, Eres32, eout, name):
    """Layernorm over features (Eres at 64x true scale); writes true-scale
    copy to eout (fp8 or fp16). Critical path: scalar pre/square -> PE sums
    -> vector stats -> two fused normalize ops producing eout; the in-place
    residual update (Eres <- 64*LN) is deferred to gpsimd off-path."""
    p256 = pools['p256']
    ones = pools['ones']
    stat = pools['stat']
    SUB = mybir.AluOpType.subtract

    # e16pre = Eres/8 = 8*t (fp16 safe: |8t| <~ 64, squares <~ 4096)
    e16pre = pools['lnp'].tile([P, KO, LL], F16, tag='e16pre', name='e16pre')
    nc.scalar.activation(e16pre[:], Eres32[:], AF.Copy, scale=0.125)
    sq16 = pools['lnp'].tile([P, KO, LL], F16, tag='sq16', name='sq16')
    nc.scalar.activation(sq16[:], e16pre[:], AF.Square)
    # sums with M=128 ones -> replicated rows; stats stay [128, LL]
    pss = p256.tile([P, LL], F32, tag='p256', name='pss')
    psq = p256.tile([P, LL], F32, tag='p256', name='psq')
    for k in range(KO):
        nc.tensor.matmul(pss[:], ones[:, :], e16pre[:, k, :],
                         start=(k == 0), stop=(k == KO - 1))
    for k in range(KO):
        nc.tensor.matmul(psq[:], ones[:, :], sq16[:, k, :],
                         start=(k == 0), stop=(k == KO - 1))
    # pss = 8*sum(t), psq = 64*sum(t^2)
    mean = stat.tile([P, LL], F32, tag='mean', name='mean')
    nc.vector.tensor_scalar_mul(mean[:], pss[:], 1.0 / (8.0 * DE))
    var = stat.tile([P, LL], F32, tag='var', name='var')
    nc.vector.tensor_scalar_mul(var[:], psq[:], 1.0 / (64.0 * DE))
    msq = stat.tile([P, LL], F32, tag='msq', name='msq')
    nc.vector.tensor_tensor(msq[:], mean[:], mean[:], MUL)
    nc.vector.tensor_tensor(var[:], var[:], msq[:], SUB)
    nc.vector.tensor_scalar_mul(var[:], var[:], float(DE) / (DE - 1))
    std = stat.tile([P, LL], F32, tag='std', name='std')
    nc.scalar.activation(std[:], var[:], AF.Sqrt, bias=pools['eps128'])
    inv = stat.tile([P, LL], F32, tag='inv', name='inv')
    nc.vector.reciprocal(inv[:], std[:])
    inv64 = stat.tile([P, LL], F32, tag='inv64', name='inv64')
    nc.vector.tensor_scalar_mul(inv64[:], inv[:], 1.0 / SR)
    # negt = -mean*inv (true-scale negms), fused in one op
    negt = stat.tile([P, LL], F32, tag='negt', name='negt')
    nc.vector.scalar_tensor_tensor(negt[:], mean[:], -1.0, inv[:], MUL, MUL)
    negms64 = stat.tile([P, LL], F32, tag='negms64', name='negms64')
    nc.vector.tensor_scalar_mul(negms64[:], negt[:], SR)
    # eout = (Eres_old * inv/64) + negt, per column half (subtile consumers)
    w16 = pools['lnp'].tile([P, KO, LL], F16, tag='w16', name='w16')
    for hf in range(2):
        sl = slice(hf * P, (hf + 1) * P)
        nc.vector.tensor_tensor(
            w16[:, :, sl], Eres32[:, :, sl],
            inv64[:, None, sl].to_broadcast((P, KO, P)), MUL)
        nc.vector.tensor_tensor(
            eout[:, :, sl], w16[:, :, sl],
            negt[:, None, sl].to_broadcast((P, KO, P)), ADD)
    # deferred: Eres <- w16*64 + negms64  (= 64 * LN(t)), off the PE path
    nc.vector.scalar_tensor_tensor(
        Eres32[:], w16[:], SR,
        negms64[:, None, :].to_broadcast((P, KO, LL)), MUL, ADD)
    tp = pools.get('tapfn')
    if tp:
        tp(f'{name}_out', eout)


def _allgather_pair(nc, e8loc, full8, agin, agout):
    """e8loc [128, KO, LL] fp8 -> pair AllGather -> full8 [128, KO, L]."""
    nc.sync.dma_start(agin[:], e8loc[:])
    nc.gpsimd.collective_compute(
        "AllGather", mybir.AluOpType.bypass,
        ins=[agin[:]], outs=[agout[:]],
        replica_groups=PAIR_GROUPS)
    nc.sync.dma_start(
        full8[:].rearrange('ki ko (r p) -> ki ko r p', r=2),
        agout[:].rearrange('r ki ko p -> ki ko r p'))


def build_program(taps=()):
    taps = set(taps)
    nc = bacc.Bacc("TRN2", target_bir_lowering=False, debug=False,
                   num_devices=N_CORES)

    # ---- dram inputs ----
    din = {}
    def dram_in(nm, shape, dt=F8):
        din[nm] = nc.dram_tensor(nm, list(shape), dt, kind="ExternalInput")
        return din[nm]

    z0f = dram_in('z0_full8', [P, KO, L])
    x0f = dram_in('x0_full8', [P, KO, L])
    z0l32 = dram_in('z0_loc32', [P, KO, LL], F32)
    x0l32 = dram_in('x0_loc32', [P, KO, LL], F32)
    z0l8 = dram_in('z0_loc8', [P, KO, LL])
    x0l8 = dram_in('x0_loc8', [P, KO, LL])
    mask_self = dram_in('mask_self', [P, 4, LL])
    for pfx, nl in (('enc', LENC), ('dec', LDEC)):
        for w in ('wq8', 'wk8'):
            dram_in(f'{pfx}_{w}', [nl, KO, P, KO, P])
        dram_in(f'{pfx}_wv8', [nl, 2, P, KO, 512])
        dram_in(f'{pfx}_wo16', [nl, KO, P, KO, P], F16)
        dram_in(f'{pfx}_w116', [nl, MO, P, KO, P], F16)
        dram_in(f'{pfx}_w216', [nl, KO, P, MO, P], F16)
    wu16 = dram_in('wu16', [P, KO, NV], F16)

    outp = nc.dram_tensor('outp', [2, P, NV], F16, kind="ExternalOutput")

    # internal dram for the K/V pair-AllGathers (4 instances in flight)
    agk = [(nc.dram_tensor(f'agkin{i}', [P, 2 * KO * LL], F16),
            nc.dram_tensor(f'agkout{i}', [2, P, 2 * KO * LL], F16))
           for i in range(4)]

    import contextlib
    with tile.TileContext(nc) as tc, contextlib.ExitStack() as octx:
        const = octx.enter_context(tc.tile_pool(name='const', bufs=1))
        ones = const.tile([P, P], F16)
        nc.vector.memset(ones[:], 1.0)
        ones64 = const.tile([P, P], F16)
        nc.vector.memset(ones64[:], 1.0 / SW)
        eps128 = const.tile([P, 1], F32)
        nc.vector.memset(eps128[:], EPS)
        msk = const.tile([P, 4, LL], F8)
        nc.sync.dma_start(msk[:], mask_self[:])
        estay = const.tile([P, KO, LL], F16)  # final decoder stream (true)

        # ================= layer phase =================
        with contextlib.ExitStack() as ctx:
            stream = ctx.enter_context(tc.tile_pool(name='stream', bufs=1))
            att = ctx.enter_context(tc.tile_pool(name='att', bufs=1))
            mlpp = ctx.enter_context(tc.tile_pool(name='mlpp', bufs=1))
            lnp = ctx.enter_context(tc.tile_pool(name='lnp', bufs=1))
            stat = ctx.enter_context(tc.tile_pool(name='stat', bufs=1))
            wqp = ctx.enter_context(tc.tile_pool(name='wqp', bufs=3))
            wkp = ctx.enter_context(tc.tile_pool(name='wkp', bufs=3))
            wvp = ctx.enter_context(tc.tile_pool(name='wvp', bufs=2))
            wop = ctx.enter_context(tc.tile_pool(name='wop', bufs=3))
            w1p = ctx.enter_context(tc.tile_pool(name='w1p', bufs=3))
            w2p = ctx.enter_context(tc.tile_pool(name='w2p', bufs=2))
            p256 = ctx.enter_context(tc.tile_pool(name='p256', bufs=5,
                                                  space='PSUM'))
            p512 = ctx.enter_context(tc.tile_pool(name='p512', bufs=3,
                                                  space='PSUM'))

            pools = dict(att=att, mlp=mlpp, lnp=lnp, p256=p256, p512=p512,
                         stat=stat, ones=ones, ones64=ones64,
                         eps128=eps128[:], wqp=wqp, wkp=wkp,
                         wvp=wvp, wop=wop, w1p=w1p, w2p=w2p)

            def tapfn(nm, t):
                if nm not in taps:
                    return
                d = nc.dram_tensor('tap_' + nm, list(t.shape),
                                   t.dtype, kind="ExternalOutput")
                nc.sync.dma_start(d[:], t[:])
            pools['tapfn'] = tapfn

            # stream tiles (enc + dec loaded up front)
            Eres = stream.tile([P, KO, LL], F32, tag='res')
            nc.sync.dma_start(Eres[:], z0l32[:])
            Zfull = stream.tile([P, KO, L], F8, tag='Zfull')
            nc.sync.dma_start(Zfull[:], z0f[:])
            eloc8 = stream.tile([P, KO, LL], F8, tag='eloc8')
            nc.sync.dma_start(eloc8[:], z0l8[:])
            EresD = stream.tile([P, KO, LL], F32, tag='res_d')
            nc.sync.dma_start(EresD[:], x0l32[:])
            Xfull = stream.tile([P, KO, L], F8, tag='Xfull')
            nc.sync.dma_start(Xfull[:], x0f[:])
            xloc8 = stream.tile([P, KO, LL], F8, tag='dloc_a')
            nc.sync.dma_start(xloc8[:], x0l8[:])

            ewq, ewk = din['enc_wq8'], din['enc_wk8']
            ewv, ewo = din['enc_wv8'], din['enc_wo16']
            dwq, dwk = din['dec_wq8'], din['dec_wk8']
            dwv, dwo = din['dec_wv8'], din['dec_wo16']

            # ---- enc0 attn (raw stream, K/V from host-provided full Z) ----
            q = _attn_q(nc, pools, eloc8, ewq[0], S_RAW)
            k, vt = _attn_kv(nc, pools, Zfull, ewk[0], ewv[0], S_RAW)
            _, y = _attn_sav(nc, pools, q, k, vt, None)
            _attn_wo(nc, pools, y, ewo[0], Eres)
            # ---- dec0 self attn (raw stream, independent of encoder) ----
            dq = _attn_q(nc, pools, xloc8, dwq[0], S_RAW)
            dk, dvt = _attn_kv(nc, pools, Xfull, dwk[0], dwv[0], S_RAW)
            # ---- enc0 ln1 + mlp (dec0self scores/AV fill the LN bubble) ----
            e16 = stream.tile([P, KO, LL], F16, tag='loc16')
            _ln(nc, pools, Eres, e16, 'e0ln1')
            _, dy = _attn_sav(nc, pools, dq, dk, dvt, msk)
            _mlp(nc, pools, Eres, e16, din['enc_w116'][0], din['enc_w216'][0])
            eloc8 = stream.tile([P, KO, LL], F8, tag='eloc8')
            _ln(nc, pools, Eres, eloc8, 'e0ln2')
            # enc1 local K/V -> AllGather; dec0self AV/Wo + enc1 Q fill it
            kl, vl = _attn_kv_local(nc, pools, eloc8, ewk[1], ewv[1], S_LN)
            _kv_ag_issue(nc, kl, vl, *agk[0])
            _attn_wo(nc, pools, dy, dwo[0], EresD)
            xloc8b = stream.tile([P, KO, LL], F8, tag='dloc_b')
            _ln(nc, pools, EresD, xloc8b, 'd0ln1')
            q = _attn_q(nc, pools, eloc8, ewq[1], S_LN)
            # ---- enc1 ----
            k, vt = _kv_ag_assemble(nc, pools, agk[0][1])
            _, y = _attn_sav(nc, pools, q, k, vt, None)
            _attn_wo(nc, pools, y, ewo[1], Eres)
            dq = _attn_q(nc, pools, xloc8b, dwq[0], S_LN)   # dec0 cross Q
            e16 = stream.tile([P, KO, LL], F16, tag='loc16')
            _ln(nc, pools, Eres, e16, 'e1ln1')
            _mlp(nc, pools, Eres, e16, din['enc_w116'][1], din['enc_w216'][1])
            eloc8 = stream.tile([P, KO, LL], F8, tag='eloc8')
            _ln(nc, pools, Eres, eloc8, 'e1ln2')   # enc-final local stream
            # cross K/V for both dec layers from the enc-final local stream
            kl, vl = _attn_kv_local(nc, pools, eloc8, dwk[0], dwv[0], S_LN)
            _kv_ag_issue(nc, kl, vl, *agk[1])
            kl, vl = _attn_kv_local(nc, pools, eloc8, dwk[1], dwv[1], S_LN)
            _kv_ag_issue(nc, kl, vl, *agk[2])

            # ---- dec0 cross ----
            dk, dvt = _kv_ag_assemble(nc, pools, agk[1][1])
            _, dy = _attn_sav(nc, pools, dq, dk, dvt, None)
            _attn_wo(nc, pools, dy, dwo[0], EresD)
            e16 = stream.tile([P, KO, LL], F16, tag='loc16')
            _ln(nc, pools, EresD, e16, 'd0ln2')
            _mlp(nc, pools, EresD, e16, din['dec_w116'][0], din['dec_w216'][0])
            xloc8c = stream.tile([P, KO, LL], F8, tag='dloc_a')
            _ln(nc, pools, EresD, xloc8c, 'd0ln3')
            # dec1 self local K/V -> AllGather; Q + cross-assemble fill it
            kl, vl = _attn_kv_local(nc, pools, xloc8c, dwk[1], dwv[1], S_LN)
            _kv_ag_issue(nc, kl, vl, *agk[3])
            dq = _attn_q(nc, pools, xloc8c, dwq[1], S_LN)
            kx, vtx = _kv_ag_assemble(nc, pools, agk[2][1],
                                      ktag='k16x', vtag='vt16x')
            # ---- dec1 self ----
            dk, dvt = _kv_ag_assemble(nc, pools, agk[3][1])
            _, dy = _attn_sav(nc, pools, dq, dk, dvt, msk)
            _attn_wo(nc, pools, dy, dwo[1], EresD)
            xloc8d = stream.tile([P, KO, LL], F8, tag='dloc_b')
            _ln(nc, pools, EresD, xloc8d, 'd1ln1')
            # ---- dec1 cross (K/V pre-gathered in k16x/vt16x) ----
            dq = _attn_q(nc, pools, xloc8d, dwq[1], S_LN)
            _, dy = _attn_sav(nc, pools, dq, kx, vtx, None)
            _attn_wo(nc, pools, dy, dwo[1], EresD)
            e16 = stream.tile([P, KO, LL], F16, tag='loc16')
            _ln(nc, pools, EresD, e16, 'd1ln2')
            _mlp(nc, pools, EresD, e16, din['dec_w116'][1], din['dec_w216'][1])
            _ln(nc, pools, EresD, estay, 'd1ln3')

        # ================= unembed phase (fully local, transposed) ========
        with contextlib.ExitStack() as ctx:
            usb = ctx.enter_context(tc.tile_pool(name='usb', bufs=1))
            wup = ctx.enter_context(tc.tile_pool(name='wup', bufs=4))
            outsb = ctx.enter_context(tc.tile_pool(name='outsb', bufs=4))
            u512 = ctx.enter_context(tc.tile_pool(name='u512', bufs=4,
                                                  space='PSUM'))

            expu = [usb.tile([P, NCH * 512], F16, tag=f'expu{tt}',
                             name=f'expu{tt}') for tt in range(2)]
            dacc = [usb.tile([P, NCH], F32, tag=f'dacc{tt}',
                             name=f'dacc{tt}') for tt in range(2)]
            for c in range(NCH):
                c0 = c * 512
                cw = 512 if c < NCH - 1 else NV - c0
                wut = wup.tile([P, KO, 512], F16, tag='wut', name='wut')
                nc.sync.dma_start(wut[:, :, :cw], wu16[:, :, c0:c0 + cw])
                for tt in range(2):
                    ps = u512.tile([P, 512], F32, tag='u512', name='psu')
                    for k in range(KO):
                        nc.tensor.matmul(ps[:, :cw],
                                         estay[:, k, tt * P:(tt + 1) * P],
                                         wut[:, k, :cw],
                                         start=(k == 0), stop=(k == KO - 1))
                    nc.scalar.activation(expu[tt][:, c0:c0 + cw], ps[:, :cw],
                                         AF.Exp,
                                         accum_out=dacc[tt][:, c:c + 1])
            binv = []
            for tt in range(2):
                dsum = usb.tile([P, 1], F32, tag=f'dsum{tt}',
                                name=f'dsum{tt}')
                nc.vector.tensor_reduce(dsum[:], dacc[tt][:],
                                        mybir.AxisListType.X,
                                        mybir.AluOpType.add)
                bi = usb.tile([P, 1], F32, tag=f'binv{tt}', name=f'binv{tt}')
                nc.vector.reciprocal(bi[:], dsum[:])
                binv.append(bi)
            # final scale+store: 2-chunk groups, scalar:vector 2:1,
            # stores spread over four DMA queues
            qrot = [nc.sync, nc.gpsimd]
            gi = 0
            for tt in range(2):
                for cc in range(0, NCH, 2):
                    c0 = cc * 512
                    w = min(1024, NV - c0)
                    ot = outsb.tile([P, 1024], F16, tag=f'ot{tt}',
                                    name=f'ot{tt}')
                    if gi % 2 == 0:
                        nc.vector.tensor_scalar_mul(ot[:, :w],
                                                    expu[tt][:, c0:c0 + w],
                                                    binv[tt][:])
                    else:
                        nc.scalar.activation(ot[:, :w],
                                             expu[tt][:, c0:c0 + w],
                                             AF.Copy, scale=binv[tt][:])
                    qrot[gi % 2].dma_start(outp[tt, :, c0:c0 + w], ot[:, :w])
                    gi += 1

    nc.compile()
    return nc


# ----------------------------------------------------------------------------
# host-side prep
# ----------------------------------------------------------------------------

def _tile_lhsT(a, mtile=P):
    """[K, M] -> [M//mtile, 128, K//128, mtile] pre-tiled stationary layout."""
    K, M = a.shape
    ko, mt = K // P, M // mtile
    return np.ascontiguousarray(
        a.reshape(ko, P, mt, mtile).transpose(2, 1, 0, 3))


def _to_kimaj(a):
    """[K, M] -> [128, K//128, M] with K = ko*128 + ki."""
    K, M = a.shape
    return np.ascontiguousarray(
        a.reshape(K // P, P, M).transpose(1, 0, 2))


def _f8(a):
    return np.clip(a, -240.0, 240.0).astype(NPF8)


def prep_inputs(inputs):
    f = lambda k: np.asarray(inputs[k], dtype=np.float32)
    We, Wp, Wu = f('We'), f('Wp'), f('Wu')
    x = np.asarray(inputs['x']).astype(np.int64)
    z = np.asarray(inputs['z']).astype(np.int64)

    shared = {}
    for pfx, nl in (('enc', LENC), ('dec', LDEC)):
        Wq, Wk, Wv = f(pfx + '_Wq'), f(pfx + '_Wk'), f(pfx + '_Wv')
        Wo, W1, W2 = f(pfx + '_Wo'), f(pfx + '_W1'), f(pfx + '_W2')
        wq, wk, wv, wo, w1, w2 = [], [], [], [], [], []
        for l in range(nl):
            qa = Wq[l].transpose(2, 0, 1).reshape(DE, H * DA) \
                * (SW * DA ** -0.5)
            ka = Wk[l].transpose(2, 0, 1).reshape(DE, H * DA) * SW
            va = Wv[l].transpose(2, 0, 1).reshape(DE, H * DA) * SW
            wq.append(_f8(_tile_lhsT(qa)))
            wk.append(_f8(_tile_lhsT(ka)))
            wv.append(_f8(_tile_lhsT(va, mtile=512)))
            wo.append(_tile_lhsT(Wo[l].T).astype(np.float16))
            w1.append(_tile_lhsT(W1[l].T).astype(np.float16))
            w2.append(_tile_lhsT(W2[l].T * SW).astype(np.float16))
        shared[f'{pfx}_wq8'] = np.stack(wq)
        shared[f'{pfx}_wk8'] = np.stack(wk)
        shared[f'{pfx}_wv8'] = np.stack(wv)
        shared[f'{pfx}_wo16'] = np.stack(wo)
        shared[f'{pfx}_w116'] = np.stack(w1)
        shared[f'{pfx}_w216'] = np.stack(w2)
    shared['wu16'] = _to_kimaj(Wu.T).astype(np.float16)

    pos = Wp[:L]  # [512, 1024]
    in_maps = []
    for c in range(N_CORES):
        b, h = c // 2, c % 2
        m = dict(shared)
        for nm, tok in (('z0', z[b]), ('x0', x[b])):
            E0 = (We[tok] + pos).T.astype(np.float32)      # [1024, 512]
            E0k = E0.reshape(KO, P, L)                     # [ko, ki, p]
            m[nm + '_full8'] = _f8(np.ascontiguousarray(
                E0k.transpose(1, 0, 2)) * SIN0)
            loc = E0k[:, :, h * LL:(h + 1) * LL].transpose(1, 0, 2)
            m[nm + '_loc32'] = np.ascontiguousarray(loc) * SR
            m[nm + '_loc8'] = _f8(np.ascontiguousarray(loc) * SIN0)
        kglob = np.arange(L)[:, None]
        qglob = (h * LL + np.arange(LL))[None, :]
        msk = (kglob <= qglob).astype(NPF8)                # [512, 256]
        m['mask_self'] = np.ascontiguousarray(
            msk.reshape(4, P, LL).transpose(1, 0, 2))
        in_maps.append(m)
    return in_maps


def assemble(results):
    """results: list of per-core dicts with 'outp' [2, 128, NV] fp16."""
    out = np.empty((4, NV, L), dtype=np.float32)
    for c, r in enumerate(results):
        b, h = c // 2, c % 2
        o = np.asarray(r['outp']).reshape(LL, NV)          # [256 tok, 32000]
        out[b, :, h * LL:(h + 1) * LL] = o.T.astype(np.float32)
    return out


def run(inputs, trace=False, taps=(), trace_kwargs=None):
    key = ('prog', tuple(sorted(taps)))
    if key not in _CACHE:
        _CACHE[key] = build_program(taps=taps)
    nc = _CACHE[key]
    in_maps = prep_inputs(inputs)
    res = run_bass_kernel_spmd(nc, in_maps, list(range(N_CORES)),
                               trace=trace, **(trace_kwargs or {}))
    return res


def kernel(**inputs):
    res = run(inputs, trace=False)
    return assemble(res.results)
